# revision 11
# baseline (speedup 1.0000x reference)
"""Causal multi-head attention (B=2, T=2048, C=1024, H=16, D=64) on 8 Trainium2
NeuronCores.

Sharding: core c = 4*b + g handles batch b (2-way data parallel) and head
group g (4-way tensor parallel over the 16 heads, 4 heads per core).  Each
core computes its heads' QKV projection, causal attention, and a partial
output projection (over its 256 feature columns); the host sums the 4
partials per batch (the "all-reduce") and adds the projection bias.

Device layout is fully transposed ("feature-major"): activations live as
[feature, token] so every matmul contraction runs over the SBUF partition
axis.  Scores are computed as S^T[s, t] = k^T.T @ q^T per head, exponentiated
unnormalized (scores are tiny: |s| < ~6, so no max-subtraction is needed),
masked causally, then AV^T = v.T @ P^T accumulates over key tiles in PSUM.
Row sums (softmax denominators) come from ones-vector matmuls; normalization
multiplies by a PE-broadcast reciprocal row.  All matmuls use float32r
(full-rate fp32 on the PE at free-dim >= 256).
"""

import numpy as np

NUM_HEADS = 16
C = 1024
D = 64
HEADS_PER_CORE = 4
N_CORES = 8
CW = 512  # t-chunk width (one fp32 PSUM bank)
ST = 128  # s-tile height (one partition block)

_PROG_CACHE = {}


def _build_program(T):
    import concourse.bacc as bacc
    import concourse.mybir as mybir
    import concourse.tile as tile
    from concourse.masks import make_identity

    dt = mybir.dt
    f32 = dt.float32
    f32r = dt.float32r
    AF = mybir.ActivationFunctionType
    ALU = mybir.AluOpType

    NCH = T // CW   # number of t-chunks
    NT = T // ST    # number of s-tiles
    KX = C // 128   # contraction tiles for the QKV projection
    JW = 3 * 2 * 128  # per-k-tile width of the fused qkv weight slab (768)

    nc = bacc.Bacc("TRN2", target_bir_lowering=False, debug=False,
                   num_devices=N_CORES)

    xT = nc.dram_tensor("xT", [C, T], f32, kind="ExternalInput").ap()
    wT = nc.dram_tensor("wT", [C, JW], f32, kind="ExternalInput").ap()
    bg = nc.dram_tensor("bg", [JW], f32, kind="ExternalInput").ap()
    pwT = nc.dram_tensor("pwT", [2 * 128, C], f32, kind="ExternalInput").ap()
    outT = nc.dram_tensor("outT", [C, T], f32, kind="ExternalOutput").ap()

    def r(ap):
        return ap.bitcast(f32r)

    with tile.TileContext(nc) as tc:
        with (
            tc.tile_pool(name="const", bufs=1) as const,
            tc.tile_pool(name="acts", bufs=1) as acts,
            tc.tile_pool(name="ptiles", bufs=4) as ptiles,
        ):
            # ---- constants / weights ----
            w_sb = const.tile([128, KX * JW], f32r, name="w_sb")
            for kc in range(KX):
                nc.sync.dma_start(out=w_sb[:, kc * JW:(kc + 1) * JW],
                                  in_=r(wT[kc * 128:(kc + 1) * 128, :]))
            pw_sb = [const.tile([64, C], f32r, name=f"pw_sb{hh}",
                                tag=f"pw_sb{hh}") for hh in range(4)]
            for hh in range(4):
                nc.sync.dma_start(out=pw_sb[hh],
                                  in_=r(pwT[hh * 64:(hh + 1) * 64, :]))
            b_sb = const.tile([128, 6], f32, name="b_sb")
            nc.sync.dma_start(out=b_sb, in_=bg.rearrange("(m p) -> p m", p=128))
            ones_f32 = const.tile([128, 64], f32, name="ones_f32")
            nc.vector.memset(ones_f32, 1.0)
            ones_sb = const.tile([128, 64], f32r, name="ones_sb")
            nc.vector.tensor_copy(out=ones_sb, in_=ones_f32)
            ident = const.tile([128, 128], f32, name="ident")
            make_identity(nc, ident)

            # persistent activations: q/k per head-pair, v (natural), av^T
            q_sb = [acts.tile([128, T], f32r, name=f"q_sb{p}", tag=f"q_sb{p}")
                    for p in range(2)]
            k_sb = [acts.tile([128, T], f32r, name=f"k_sb{p}", tag=f"k_sb{p}")
                    for p in range(2)]
            v_sb = [acts.tile([128, T], f32r, name=f"v_sb{p}", tag=f"v_sb{p}")
                    for p in range(2)]
            av_sb = [acts.tile([64, T], f32r, name=f"av_sb{hh}",
                              tag=f"av_sb{hh}") for hh in range(4)]
            # ---- phase 1: fused QKV projection (outputs transposed) ----
            with tc.tile_pool(name="xslab", bufs=2) as xslab, \
                 tc.tile_pool(name="vtsb", bufs=1) as vtsb, \
                 tc.tile_pool(name="qkv_ps", bufs=2, space="PSUM") as qkv_psp, \
                 tc.tile_pool(name="vt_ps", bufs=2, space="PSUM") as vt_psp:
                vT_sb = [vtsb.tile([128, T], f32, name=f"vT_sb{p}",
                                   tag=f"vT_sb{p}") for p in range(2)]
                qkv_dst = q_sb + k_sb + vT_sb  # m-tiles: q0 q1 k0 k1 v0 v1
                for ch in range(NCH):
                    slab = xslab.tile([128, KX * CW], f32r, name="slab",
                                      tag="slab")
                    for kc in range(KX):
                        nc.sync.dma_start(
                            out=slab[:, kc * CW:(kc + 1) * CW],
                            in_=r(xT[kc * 128:(kc + 1) * 128,
                                     ch * CW:(ch + 1) * CW]))
                    for mt in range(6):
                        ps = qkv_psp.tile([128, CW], f32, name="qkv_ps",
                                          tag="qkv_ps")
                        for kc in range(KX):
                            nc.tensor.matmul(
                                ps,
                                lhsT=w_sb[:, kc * JW + mt * 128:
                                          kc * JW + (mt + 1) * 128],
                                rhs=slab[:, kc * CW:(kc + 1) * CW],
                                start=(kc == 0), stop=(kc == KX - 1))
                        nc.vector.tensor_scalar_add(
                            out=qkv_dst[mt][:, ch * CW:(ch + 1) * CW],
                            in0=ps, scalar1=b_sb[:, mt:mt + 1])

                # ---- phase 1b: transpose v^T -> v (natural [s, d]) ----
                for p in range(2):
                    for i in range(NT):
                        tp = vt_psp.tile([128, 128], f32, name="vt_ps",
                                         tag="vt_ps")
                        nc.tensor.transpose(
                            tp, vT_sb[p][:, i * 128:(i + 1) * 128], ident)
                        nc.vector.tensor_copy(
                            out=v_sb[p][:, i * 128:(i + 1) * 128], in_=tp)

            # ---- phase 2: causal attention per head pair ----
            with tc.tile_pool(name="small", bufs=2) as small, \
                 tc.tile_pool(name="sc_ps", bufs=2, space="PSUM") as sc_psp, \
                 tc.tile_pool(name="av_ps", bufs=1, space="PSUM") as av_psp, \
                 tc.tile_pool(name="dn_ps", bufs=2, space="PSUM") as dn_psp:
                for p in range(2):
                    for ch in range(NCH):
                        n_i = 4 * ch + 4  # s-tiles 0 .. 4*ch+3
                        av_ps = [av_psp.tile([64, CW], f32,
                                             name=f"av_ps{h2}",
                                             tag=f"av{h2}")
                                 for h2 in range(2)]
                        dn_ps = [dn_psp.tile([1, CW], f32, name=f"dn_ps{h2}",
                                             tag="dnbc") for h2 in range(2)]
                        for i in range(n_i):
                            m = i - 4 * ch
                            col0 = 128 * m if m > 0 else 0
                            sc_ps = sc_psp.tile([128, 2 * CW], f32,
                                                name="sc_ps", tag="sc_ps")
                            for h2 in range(2):
                                nc.tensor.matmul(
                                    sc_ps[:, h2 * CW + col0:(h2 + 1) * CW],
                                    lhsT=k_sb[p][h2 * 64:(h2 + 1) * 64,
                                                 i * 128:(i + 1) * 128],
                                    rhs=q_sb[p][h2 * 64:(h2 + 1) * 64,
                                                ch * CW + col0:
                                                (ch + 1) * CW],
                                    start=True, stop=True)
                            p_sb = ptiles.tile([128, 2 * CW], f32r,
                                               name="p_sb", tag="p_sb")
                            sc3 = sc_ps.rearrange("q (h w) -> q h w", h=2)
                            p3 = p_sb.rearrange("q (h w) -> q h w", h=2)
                            nc.scalar.activation(
                                out=p3[:, :, col0:CW],
                                in_=sc3[:, :, col0:CW], func=AF.Exp)
                            if m >= 0:
                                blk = p3[:, :, col0:col0 + 128]
                                nc.gpsimd.affine_select(
                                    out=blk, in_=blk,
                                    pattern=[[0, 2], [1, 128]],
                                    channel_multiplier=-1, base=0,
                                    compare_op=ALU.is_ge, fill=0.0)
                            last = (i == n_i - 1)
                            for h2 in range(2):
                                nc.tensor.matmul(
                                    av_ps[h2][:, col0:CW],
                                    lhsT=v_sb[p][:, i * 128 + h2 * 64:
                                                 i * 128 + (h2 + 1) * 64],
                                    rhs=p_sb[:, h2 * CW + col0:
                                             (h2 + 1) * CW],
                                    start=(i == 0), stop=last)
                            for h2 in range(2):
                                nc.tensor.matmul(
                                    dn_ps[h2][:, col0:CW],
                                    lhsT=ones_sb[:, 0:1],
                                    rhs=p_sb[:, h2 * CW + col0:
                                             (h2 + 1) * CW],
                                    start=(i == 0), stop=last)
                        # normalization: recip rows, PE-broadcast, multiply
                        for h2 in range(2):
                            rc = small.tile([1, CW], f32r, name=f"rc{h2}",
                                            tag="rc")
                            with nc.allow_low_precision(
                                    reason="softmax denominator reciprocal"):
                                nc.vector.reciprocal(out=rc, in_=dn_ps[h2])
                            bc_ps = dn_psp.tile([64, CW], f32,
                                                name=f"bc_ps{h2}",
                                                tag="dnbc")
                            nc.tensor.matmul(
                                bc_ps, lhsT=ones_sb[0:1, 0:64],
                                rhs=rc, start=True, stop=True)
                            bc_sb = small.tile([64, CW], f32,
                                               name=f"bc_sb{h2}",
                                               tag="bc_sb")
                            nc.vector.tensor_copy(out=bc_sb, in_=bc_ps)
                            nc.vector.tensor_mul(
                                av_sb[2 * p + h2][:, ch * CW:(ch + 1) * CW],
                                av_ps[h2], bc_sb)

            # ---- phase 3: partial output projection ----
            with tc.tile_pool(name="prstage", bufs=4) as prstage, \
                 tc.tile_pool(name="pr_ps", bufs=4, space="PSUM") as pr_psp:
                for mt in range(8):
                    for ch in range(NCH):
                        ps = pr_psp.tile([128, CW], f32, name="pr_ps",
                                         tag="pr_ps")
                        for hh in range(4):
                            nc.tensor.matmul(
                                ps,
                                lhsT=pw_sb[hh][:, mt * 128:(mt + 1) * 128],
                                rhs=av_sb[hh][:, ch * CW:(ch + 1) * CW],
                                start=(hh == 0), stop=(hh == 3))
                        stage = prstage.tile([128, CW], f32, name="pr_stage",
                                            tag="pr_stage")
                        nc.vector.tensor_copy(out=stage, in_=ps)
                        nc.sync.dma_start(
                            out=outT[mt * 128:(mt + 1) * 128,
                                     ch * CW:(ch + 1) * CW],
                            in_=stage)

    nc.compile()
    return nc


def _get_program(T):
    if T not in _PROG_CACHE:
        _PROG_CACHE[T] = _build_program(T)
    return _PROG_CACHE[T]


def _prep_inputs(x, attn_w, attn_b, proj_w):
    """Host-side sharding/layout prep. Returns per-core in_maps."""
    B, T, C_ = x.shape
    scale = 1.0 / np.sqrt(D)
    xTs = [np.ascontiguousarray(x[b].T) for b in range(B)]
    in_maps = []
    for c in range(N_CORES):
        b, g = divmod(c, 4)
        r0 = 256 * g
        wq = attn_w[r0:r0 + 256] * scale
        wk = attn_w[C_ + r0:C_ + r0 + 256]
        wv = attn_w[2 * C_ + r0:2 * C_ + r0 + 256]
        wgT = np.ascontiguousarray(np.concatenate([wq, wk, wv], axis=0).T)
        bgv = np.concatenate([attn_b[r0:r0 + 256] * scale,
                              attn_b[C_ + r0:C_ + r0 + 256],
                              attn_b[2 * C_ + r0:2 * C_ + r0 + 256]])
        pwTg = np.ascontiguousarray(proj_w[:, r0:r0 + 256].T)
        in_maps.append({
            "xT": xTs[b],
            "wT": wgT.astype(np.float32),
            "bg": bgv.astype(np.float32),
            "pwT": pwTg.astype(np.float32),
        })
    return in_maps


def _gather(results, proj_b, B, T):
    out = np.empty((B, T, C), dtype=np.float32)
    for b in range(B):
        acc = results[4 * b]["outT"].astype(np.float32).copy()
        for g in range(1, 4):
            acc += results[4 * b + g]["outT"]
        out[b] = acc.T + proj_b[None, :]
    return out


def kernel(x, attn_w, attn_b, proj_w, proj_b, _trace=False):
    from concourse.bass_utils import run_bass_kernel_spmd
    x = np.asarray(x, dtype=np.float32)
    attn_w = np.asarray(attn_w, dtype=np.float32)
    attn_b = np.asarray(attn_b, dtype=np.float32)
    proj_w = np.asarray(proj_w, dtype=np.float32)
    proj_b = np.asarray(proj_b, dtype=np.float32)

    B, T, _ = x.shape
    nc = _get_program(T)
    in_maps = _prep_inputs(x, attn_w, attn_b, proj_w)
    res = run_bass_kernel_spmd(nc, in_maps, core_ids=list(range(N_CORES)),
                               trace=_trace)
    out = _gather(res.results, proj_b, B, T)
    if _trace:
        return out, res
    return out


# revision 12
# speedup vs baseline: 1.1599x; 1.1599x over previous
"""Causal multi-head attention (B=2, T=2048, C=1024, H=16, D=64) on 8 Trainium2
NeuronCores.

Sharding: core c = 4*b + g handles batch b (2-way data parallel) and head
group g (4-way tensor parallel over the 16 heads, 4 heads per core).  Each
core computes its heads' QKV projection, causal attention, and a partial
output projection (over its 256 feature columns); the host sums the 4
partials per batch (the "all-reduce") and adds the projection bias.

Device layout is fully transposed ("feature-major"): activations live as
[feature, token] so every matmul contraction runs over the SBUF partition
axis.  Scores are computed as S^T[s, t] = k^T.T @ q^T per head, exponentiated
unnormalized (scores are tiny: |s| < ~6, so no max-subtraction is needed),
masked causally, then AV^T = v.T @ P^T accumulates over key tiles in PSUM.
Row sums (softmax denominators) come from ones-vector matmuls; normalization
multiplies by a PE-broadcast reciprocal row.  All matmuls use float32r
(full-rate fp32 on the PE at free-dim >= 256).
"""

import numpy as np

NUM_HEADS = 16
C = 1024
D = 64
HEADS_PER_CORE = 4
N_CORES = 8
CW = 512  # t-chunk width (one fp32 PSUM bank)
ST = 128  # s-tile height (one partition block)

_PROG_CACHE = {}


def _build_program(T):
    import concourse.bacc as bacc
    import concourse.mybir as mybir
    import concourse.tile as tile
    from concourse.masks import make_identity

    dt = mybir.dt
    f32 = dt.float32
    f32r = dt.float32r
    AF = mybir.ActivationFunctionType
    ALU = mybir.AluOpType

    NCH = T // CW   # number of t-chunks
    NT = T // ST    # number of s-tiles
    KX = C // 128   # contraction tiles for the QKV projection
    JW = 3 * 2 * 128  # per-k-tile width of the fused qkv weight slab (768)

    nc = bacc.Bacc("TRN2", target_bir_lowering=False, debug=False,
                   num_devices=N_CORES)

    xT = nc.dram_tensor("xT", [C, T], f32, kind="ExternalInput").ap()
    wT = nc.dram_tensor("wT", [C, JW], f32, kind="ExternalInput").ap()
    bg = nc.dram_tensor("bg", [JW], f32, kind="ExternalInput").ap()
    pwT = nc.dram_tensor("pwT", [2 * 128, C], f32, kind="ExternalInput").ap()
    outT = nc.dram_tensor("outT", [C, T], f32, kind="ExternalOutput").ap()

    def r(ap):
        return ap.bitcast(f32r)

    with tile.TileContext(nc) as tc:
        with (
            tc.tile_pool(name="const", bufs=1) as const,
            tc.tile_pool(name="acts", bufs=1) as acts,
            tc.tile_pool(name="ptiles", bufs=4) as ptiles,
        ):
            # ---- constants / weights ----
            w_sb = const.tile([128, KX * JW], f32r, name="w_sb")
            for kc in range(KX):
                nc.sync.dma_start(out=w_sb[:, kc * JW:(kc + 1) * JW],
                                  in_=r(wT[kc * 128:(kc + 1) * 128, :]))
            pw_sb = [const.tile([64, C], f32r, name=f"pw_sb{hh}",
                                tag=f"pw_sb{hh}") for hh in range(4)]
            for hh in range(4):
                nc.sync.dma_start(out=pw_sb[hh],
                                  in_=r(pwT[hh * 64:(hh + 1) * 64, :]))
            b_sb = const.tile([128, 6], f32, name="b_sb")
            nc.sync.dma_start(out=b_sb, in_=bg.rearrange("(m p) -> p m", p=128))
            ones_f32 = const.tile([128, 64], f32, name="ones_f32")
            nc.vector.memset(ones_f32, 1.0)
            ones_sb = const.tile([128, 64], f32r, name="ones_sb")
            nc.vector.tensor_copy(out=ones_sb, in_=ones_f32)
            ident = const.tile([128, 128], f32, name="ident")
            make_identity(nc, ident)

            # persistent activations: q/k per head-pair, v (natural), av^T
            q_sb = [acts.tile([128, T], f32r, name=f"q_sb{p}", tag=f"q_sb{p}")
                    for p in range(2)]
            k_sb = [acts.tile([128, T], f32r, name=f"k_sb{p}", tag=f"k_sb{p}")
                    for p in range(2)]
            v_sb = [acts.tile([128, (T // 128) * 130], f32r,
                              name=f"v_sb{p}", tag=f"v_sb{p}")
                    for p in range(2)]
            av_sb = [acts.tile([64, T], f32r, name=f"av_sb{hh}",
                              tag=f"av_sb{hh}") for hh in range(4)]
            # ---- phase 1: fused QKV projection (outputs transposed) ----
            with tc.tile_pool(name="xslab", bufs=2) as xslab, \
                 tc.tile_pool(name="vtsb", bufs=1) as vtsb, \
                 tc.tile_pool(name="qkv_ps", bufs=2, space="PSUM") as qkv_psp, \
                 tc.tile_pool(name="vt_ps", bufs=2, space="PSUM") as vt_psp:
                vT_sb = [vtsb.tile([128, T], f32, name=f"vT_sb{p}",
                                   tag=f"vT_sb{p}") for p in range(2)]
                qkv_dst = q_sb + k_sb + vT_sb  # m-tiles: q0 q1 k0 k1 v0 v1
                for ch in range(NCH):
                    slab = xslab.tile([128, KX * CW], f32r, name="slab",
                                      tag="slab")
                    for kc in range(KX):
                        nc.sync.dma_start(
                            out=slab[:, kc * CW:(kc + 1) * CW],
                            in_=r(xT[kc * 128:(kc + 1) * 128,
                                     ch * CW:(ch + 1) * CW]))
                    for mt in range(6):
                        ps = qkv_psp.tile([128, CW], f32, name="qkv_ps",
                                          tag="qkv_ps")
                        for kc in range(KX):
                            nc.tensor.matmul(
                                ps,
                                lhsT=w_sb[:, kc * JW + mt * 128:
                                          kc * JW + (mt + 1) * 128],
                                rhs=slab[:, kc * CW:(kc + 1) * CW],
                                start=(kc == 0), stop=(kc == KX - 1))
                        nc.vector.tensor_scalar_add(
                            out=qkv_dst[mt][:, ch * CW:(ch + 1) * CW],
                            in0=ps, scalar1=b_sb[:, mt:mt + 1])

                # ---- phase 1b: transpose v^T -> v (natural [s, d]) ----
                for p in range(2):
                    for i in range(NT):
                        tp = vt_psp.tile([128, 128], f32, name="vt_ps",
                                         tag="vt_ps")
                        nc.tensor.transpose(
                            tp, vT_sb[p][:, i * 128:(i + 1) * 128], ident)
                        dst = v_sb[p][:, 130 * i:130 * i + 130]
                        nc.vector.tensor_copy(
                            out=dst.rearrange("q (h c) -> q h c",
                                              h=2)[:, :, 0:64],
                            in_=tp.rearrange("q (h c) -> q h c", h=2))
                        nc.vector.tensor_copy(
                            out=v_sb[p][:, 130 * i + 64:130 * i + 130:65],
                            in_=ones_f32[:, 0:2])

            # ---- phase 2: causal attention per head pair ----
            with tc.tile_pool(name="small", bufs=2) as small, \
                 tc.tile_pool(name="sc_ps", bufs=2, space="PSUM") as sc_psp, \
                 tc.tile_pool(name="av_ps", bufs=1, space="PSUM") as av_psp, \
                 tc.tile_pool(name="dn_ps", bufs=2, space="PSUM") as dn_psp:
                for p in range(2):
                    for ch in range(NCH):
                        n_i = 4 * ch + 4  # s-tiles 0 .. 4*ch+3
                        av_ps = [av_psp.tile([65, CW], f32,
                                             name=f"av_ps{h2}",
                                             tag=f"av{h2}")
                                 for h2 in range(2)]
                        for i in range(n_i):
                            m = i - 4 * ch
                            col0 = 128 * m if m > 0 else 0
                            sc_ps = sc_psp.tile([128, 2 * CW], f32,
                                                name="sc_ps", tag="sc_ps")
                            for h2 in range(2):
                                nc.tensor.matmul(
                                    sc_ps[:, h2 * CW + col0:(h2 + 1) * CW],
                                    lhsT=k_sb[p][h2 * 64:(h2 + 1) * 64,
                                                 i * 128:(i + 1) * 128],
                                    rhs=q_sb[p][h2 * 64:(h2 + 1) * 64,
                                                ch * CW + col0:
                                                (ch + 1) * CW],
                                    start=True, stop=True)
                            p_sb = ptiles.tile([128, 2 * CW], f32r,
                                               name="p_sb", tag="p_sb")
                            sc3 = sc_ps.rearrange("q (h w) -> q h w", h=2)
                            p3 = p_sb.rearrange("q (h w) -> q h w", h=2)
                            nc.scalar.activation(
                                out=p3[:, :, col0:CW],
                                in_=sc3[:, :, col0:CW], func=AF.Exp)
                            if m >= 0:
                                blk = p3[:, :, col0:col0 + 128]
                                nc.gpsimd.affine_select(
                                    out=blk, in_=blk,
                                    pattern=[[0, 2], [1, 128]],
                                    channel_multiplier=-1, base=0,
                                    compare_op=ALU.is_ge, fill=0.0)
                            last = (i == n_i - 1)
                            for h2 in range(2):
                                nc.tensor.matmul(
                                    av_ps[h2][:, col0:CW],
                                    lhsT=v_sb[p][:, 130 * i + 65 * h2:
                                                 130 * i + 65 * h2 + 65],
                                    rhs=p_sb[:, h2 * CW + col0:
                                             (h2 + 1) * CW],
                                    start=(i == 0), stop=last)
                        # epilogue: evict av+denom to SBUF fast (frees
                        # PSUM); reciprocal + broadcast + normalize trail
                        # off the critical path.
                        for h2 in range(2):
                            av_un = small.tile([65, CW], f32,
                                               name=f"av_un{h2}",
                                               tag=f"av_un{h2}", bufs=2)
                            nc.vector.tensor_copy(out=av_un, in_=av_ps[h2])
                            dn_sb = small.tile([1, CW], f32,
                                               name=f"dn_sb{h2}",
                                               tag=f"dn_sb{h2}", bufs=2)
                            nc.sync.dma_start(out=dn_sb,
                                              in_=av_un[64:65, :])
                            rc = small.tile([1, CW], f32r, name=f"rc{h2}",
                                            tag="rc", bufs=4)
                            with nc.allow_low_precision(
                                    reason="softmax denominator reciprocal"):
                                nc.vector.reciprocal(out=rc, in_=dn_sb)
                            bc_ps = dn_psp.tile([64, CW], f32,
                                                name=f"bc_ps{h2}",
                                                tag="dnbc")
                            nc.tensor.matmul(
                                bc_ps, lhsT=ones_sb[0:1, 0:64],
                                rhs=rc, start=True, stop=True)
                            nc.vector.tensor_mul(
                                av_sb[2 * p + h2][:, ch * CW:(ch + 1) * CW],
                                av_un[0:64, :], bc_ps)

            # ---- phase 3: partial output projection ----
            with tc.tile_pool(name="prstage", bufs=4) as prstage, \
                 tc.tile_pool(name="pr_ps", bufs=4, space="PSUM") as pr_psp:
                for mt in range(8):
                    for ch in range(NCH):
                        ps = pr_psp.tile([128, CW], f32, name="pr_ps",
                                         tag="pr_ps")
                        for hh in range(4):
                            nc.tensor.matmul(
                                ps,
                                lhsT=pw_sb[hh][:, mt * 128:(mt + 1) * 128],
                                rhs=av_sb[hh][:, ch * CW:(ch + 1) * CW],
                                start=(hh == 0), stop=(hh == 3))
                        stage = prstage.tile([128, CW], f32, name="pr_stage",
                                            tag="pr_stage")
                        nc.vector.tensor_copy(out=stage, in_=ps)
                        nc.sync.dma_start(
                            out=outT[mt * 128:(mt + 1) * 128,
                                     ch * CW:(ch + 1) * CW],
                            in_=stage)

    nc.compile()
    return nc


def _get_program(T):
    if T not in _PROG_CACHE:
        _PROG_CACHE[T] = _build_program(T)
    return _PROG_CACHE[T]


def _prep_inputs(x, attn_w, attn_b, proj_w):
    """Host-side sharding/layout prep. Returns per-core in_maps."""
    B, T, C_ = x.shape
    scale = 1.0 / np.sqrt(D)
    xTs = [np.ascontiguousarray(x[b].T) for b in range(B)]
    in_maps = []
    for c in range(N_CORES):
        b, g = divmod(c, 4)
        r0 = 256 * g
        wq = attn_w[r0:r0 + 256] * scale
        wk = attn_w[C_ + r0:C_ + r0 + 256]
        wv = attn_w[2 * C_ + r0:2 * C_ + r0 + 256]
        wgT = np.ascontiguousarray(np.concatenate([wq, wk, wv], axis=0).T)
        bgv = np.concatenate([attn_b[r0:r0 + 256] * scale,
                              attn_b[C_ + r0:C_ + r0 + 256],
                              attn_b[2 * C_ + r0:2 * C_ + r0 + 256]])
        pwTg = np.ascontiguousarray(proj_w[:, r0:r0 + 256].T)
        in_maps.append({
            "xT": xTs[b],
            "wT": wgT.astype(np.float32),
            "bg": bgv.astype(np.float32),
            "pwT": pwTg.astype(np.float32),
        })
    return in_maps


def _gather(results, proj_b, B, T):
    out = np.empty((B, T, C), dtype=np.float32)
    for b in range(B):
        acc = results[4 * b]["outT"].astype(np.float32).copy()
        for g in range(1, 4):
            acc += results[4 * b + g]["outT"]
        out[b] = acc.T + proj_b[None, :]
    return out


def kernel(x, attn_w, attn_b, proj_w, proj_b, _trace=False):
    from concourse.bass_utils import run_bass_kernel_spmd
    x = np.asarray(x, dtype=np.float32)
    attn_w = np.asarray(attn_w, dtype=np.float32)
    attn_b = np.asarray(attn_b, dtype=np.float32)
    proj_w = np.asarray(proj_w, dtype=np.float32)
    proj_b = np.asarray(proj_b, dtype=np.float32)

    B, T, _ = x.shape
    nc = _get_program(T)
    in_maps = _prep_inputs(x, attn_w, attn_b, proj_w)
    res = run_bass_kernel_spmd(nc, in_maps, core_ids=list(range(N_CORES)),
                               trace=_trace)
    out = _gather(res.results, proj_b, B, T)
    if _trace:
        return out, res
    return out


# revision 13
# speedup vs baseline: 1.2751x; 1.0993x over previous
"""Causal multi-head attention (B=2, T=2048, C=1024, H=16, D=64) on 8 Trainium2
NeuronCores.

Sharding: core c = 4*b + g handles batch b (2-way data parallel) and head
group g (4-way tensor parallel over the 16 heads, 4 heads per core).  Each
core computes its heads' QKV projection, causal attention, and a partial
output projection (over its 256 feature columns); the host sums the 4
partials per batch (the "all-reduce") and adds the projection bias.

Device layout is fully transposed ("feature-major"): activations live as
[feature, token] so every matmul contraction runs over the SBUF partition
axis.  Scores are computed as S^T[s, t] = k^T.T @ q^T per head (two heads
packed into the 128 PE rows), exponentiated unnormalized (scores are tiny:
|s| < ~6, so no max-subtraction is needed), masked causally with
affine_select, then AV^T accumulates over key tiles in PSUM with a ones
column appended to V so the softmax denominator falls out of the same
matmul.  Normalization (reciprocal + PE-row-broadcast + multiply) trails
off the critical path.  Matmul operands are bf16 (cast on host / at PSUM
eviction); accumulation stays fp32 in PSUM.
"""

import numpy as np
import ml_dtypes

NUM_HEADS = 16
C = 1024
D = 64
N_CORES = 8
CW = 512   # t-chunk width (one fp32 PSUM bank)
ST = 128   # s-tile height (one partition block)

_PROG_CACHE = {}


def _build_program(T):
    import concourse.bacc as bacc
    import concourse.mybir as mybir
    import concourse.tile as tile
    from concourse.masks import make_identity

    dt = mybir.dt
    f32 = dt.float32
    f32r = dt.float32r
    bf16 = dt.bfloat16
    AF = mybir.ActivationFunctionType
    ALU = mybir.AluOpType

    NCH = T // CW   # number of t-chunks
    NT = T // ST    # number of s-tiles
    KX = C // 128   # contraction tiles for the QKV projection
    JW = 3 * 2 * 128  # per-k-tile width of the fused qkv weight slab (768)

    nc = bacc.Bacc("TRN2", target_bir_lowering=False, debug=False,
                   num_devices=N_CORES)

    xT = nc.dram_tensor("xT", [C, T], bf16, kind="ExternalInput").ap()
    wT = nc.dram_tensor("wT", [C, JW], bf16, kind="ExternalInput").ap()
    bg = nc.dram_tensor("bg", [JW], f32, kind="ExternalInput").ap()
    pwT = nc.dram_tensor("pwT", [2 * 128, C], bf16, kind="ExternalInput").ap()
    outT = nc.dram_tensor("outT", [C, T], f32, kind="ExternalOutput").ap()

    with tile.TileContext(nc) as tc:
        with (
            tc.tile_pool(name="const", bufs=1) as const,
            tc.tile_pool(name="acts", bufs=1) as acts,
            tc.tile_pool(name="ptiles", bufs=4) as ptiles,
        ):
            # ---- qkv weights first: the first matmul waits on block 0 ----
            w_sb = [const.tile([128, JW], bf16, name=f"w_sb{kc}",
                               tag=f"w_sb{kc}") for kc in range(KX)]
            for kc in range(KX):
                nc.sync.dma_start(out=w_sb[kc],
                                  in_=wT[kc * 128:(kc + 1) * 128, :])
            b_sb = const.tile([128, 6], f32, name="b_sb")
            nc.sync.dma_start(out=b_sb, in_=bg.rearrange("(m p) -> p m", p=128))
            ones_f32 = const.tile([128, 64], f32, name="ones_f32")
            nc.vector.memset(ones_f32, 1.0)
            ones_bf = const.tile([128, 2], bf16, name="ones_bf")
            nc.vector.tensor_copy(out=ones_bf, in_=ones_f32[:, 0:2])
            ones_r = const.tile([1, 64], f32r, name="ones_r")
            nc.vector.tensor_copy(out=ones_r, in_=ones_f32[0:1, :])
            ident = const.tile([128, 128], f32, name="ident_f32")
            make_identity(nc, ident)
            ident_bf = const.tile([128, 128], bf16, name="ident_bf")
            nc.vector.tensor_copy(out=ident_bf, in_=ident)

            # persistent activations: q/k per head-pair, v (natural), av^T
            q_sb = [acts.tile([128, T], bf16, name=f"q_sb{p}", tag=f"q_sb{p}")
                    for p in range(2)]
            k_sb = [acts.tile([128, T], bf16, name=f"k_sb{p}", tag=f"k_sb{p}")
                    for p in range(2)]
            v_sb = [acts.tile([128, NT * 130], bf16,
                              name=f"v_sb{p}", tag=f"v_sb{p}")
                    for p in range(2)]
            av_sb = [acts.tile([64, T], bf16, name=f"av_sb{hh}",
                               tag=f"av_sb{hh}") for hh in range(4)]

            # ---- phase 1: fused QKV projection (outputs transposed) ----
            with tc.tile_pool(name="xslab", bufs=16) as xslab, \
                 tc.tile_pool(name="vtsb", bufs=1) as vtsb, \
                 tc.tile_pool(name="qkv_ps", bufs=2, space="PSUM") as qkv_psp, \
                 tc.tile_pool(name="vt_ps", bufs=2, space="PSUM") as vt_psp:
                vT_sb = [vtsb.tile([128, T], bf16, name=f"vT_sb{p}",
                                   tag=f"vT_sb{p}") for p in range(2)]
                qkv_dst = q_sb + k_sb + vT_sb  # m-tiles: q0 q1 k0 k1 v0 v1
                for ch in range(NCH):
                    slabs = []
                    for kc in range(KX):
                        sl = xslab.tile([128, CW], bf16, name=f"slab{kc}",
                                        tag="slab")
                        nc.sync.dma_start(
                            out=sl, in_=xT[kc * 128:(kc + 1) * 128,
                                           ch * CW:(ch + 1) * CW])
                        slabs.append(sl)
                    for mt in range(6):
                        ps = qkv_psp.tile([128, CW], f32, name="qkv_ps",
                                          tag="qkv_ps")
                        for kc in range(KX):
                            nc.tensor.matmul(
                                ps,
                                lhsT=w_sb[kc][:, mt * 128:(mt + 1) * 128],
                                rhs=slabs[kc],
                                start=(kc == 0), stop=(kc == KX - 1))
                        nc.vector.tensor_scalar_add(
                            out=qkv_dst[mt][:, ch * CW:(ch + 1) * CW],
                            in0=ps, scalar1=b_sb[:, mt:mt + 1])

                # ---- phase 1b: transpose v^T -> v_aug ([s, d|1] bf16) ----
                for p in range(2):
                    for i in range(NT):
                        tp = vt_psp.tile([128, 128], bf16, name="vt_ps",
                                         tag="vt_ps")
                        nc.tensor.transpose(
                            tp, vT_sb[p][:, i * 128:(i + 1) * 128], ident_bf)
                        dst = v_sb[p][:, 130 * i:130 * i + 130]
                        nc.vector.tensor_copy(
                            out=dst.rearrange("q (h c) -> q h c",
                                              h=2)[:, :, 0:64],
                            in_=tp.rearrange("q (h c) -> q h c", h=2))
                        nc.vector.tensor_copy(
                            out=v_sb[p][:, 130 * i + 64:130 * i + 130:65],
                            in_=ones_bf)

            # ---- phase 2: causal attention per head pair ----
            with tc.tile_pool(name="small", bufs=2) as small, \
                 tc.tile_pool(name="sc_ps", bufs=2, space="PSUM") as sc_psp, \
                 tc.tile_pool(name="av_ps", bufs=1, space="PSUM") as av_psp, \
                 tc.tile_pool(name="bc_ps", bufs=2, space="PSUM") as bc_psp:
                for p in range(2):
                    for ch in range(NCH):
                        n_i = 4 * ch + 4  # s-tiles 0 .. 4*ch+3
                        av_ps = [av_psp.tile([65, CW], f32,
                                             name=f"av_ps{h2}",
                                             tag=f"av{h2}")
                                 for h2 in range(2)]
                        for i in range(n_i):
                            m = i - 4 * ch
                            col0 = 128 * m if m > 0 else 0
                            sc_ps = sc_psp.tile([128, 2 * CW], f32,
                                                name="sc_ps", tag="sc_ps")
                            for h2 in range(2):
                                nc.tensor.matmul(
                                    sc_ps[:, h2 * CW + col0:(h2 + 1) * CW],
                                    lhsT=k_sb[p][h2 * 64:(h2 + 1) * 64,
                                                 i * 128:(i + 1) * 128],
                                    rhs=q_sb[p][h2 * 64:(h2 + 1) * 64,
                                                ch * CW + col0:
                                                (ch + 1) * CW],
                                    start=True, stop=True)
                            p_sb = ptiles.tile([128, 2 * CW], bf16,
                                               name="p_sb", tag="p_sb")
                            sc3 = sc_ps.rearrange("q (h w) -> q h w", h=2)
                            p3 = p_sb.rearrange("q (h w) -> q h w", h=2)
                            nc.scalar.activation(
                                out=p3[:, :, col0:CW],
                                in_=sc3[:, :, col0:CW], func=AF.Exp)
                            if m >= 0:
                                blk = p3[:, :, col0:col0 + 128]
                                nc.gpsimd.affine_select(
                                    out=blk, in_=blk,
                                    pattern=[[0, 2], [1, 128]],
                                    channel_multiplier=-1, base=0,
                                    compare_op=ALU.is_ge, fill=0.0)
                            last = (i == n_i - 1)
                            for h2 in range(2):
                                nc.tensor.matmul(
                                    av_ps[h2][:, col0:CW],
                                    lhsT=v_sb[p][:, 130 * i + 65 * h2:
                                                 130 * i + 65 * h2 + 65],
                                    rhs=p_sb[:, h2 * CW + col0:
                                             (h2 + 1) * CW],
                                    start=(i == 0), stop=last)
                        # epilogue: evict av+denominator to SBUF fast (frees
                        # PSUM); reciprocal + broadcast + normalize trail off
                        # the critical path.
                        for h2 in range(2):
                            av_un = small.tile([65, CW], f32,
                                               name=f"av_un{h2}",
                                               tag=f"av_un{h2}", bufs=2)
                            nc.vector.tensor_copy(out=av_un, in_=av_ps[h2])
                            dn_sb = small.tile([1, CW], f32,
                                               name=f"dn_sb{h2}",
                                               tag=f"dn_sb{h2}", bufs=2)
                            nc.sync.dma_start(out=dn_sb,
                                              in_=av_un[64:65, :])
                            rc = small.tile([1, CW], f32r, name=f"rc{h2}",
                                            tag="rc", bufs=4)
                            with nc.allow_low_precision(
                                    reason="softmax denominator reciprocal"):
                                nc.vector.reciprocal(out=rc, in_=dn_sb)
                            bc_ps = bc_psp.tile([64, CW], f32,
                                                name=f"bc_ps{h2}",
                                                tag="bc")
                            nc.tensor.matmul(
                                bc_ps, lhsT=ones_r, rhs=rc,
                                start=True, stop=True)
                            nc.vector.tensor_mul(
                                av_sb[2 * p + h2][:, ch * CW:(ch + 1) * CW],
                                av_un[0:64, :], bc_ps)

            # ---- phase 3: partial output projection ----
            with tc.tile_pool(name="prstage", bufs=4) as prstage, \
                 tc.tile_pool(name="pwpool", bufs=1) as pwpool, \
                 tc.tile_pool(name="pr_ps", bufs=4, space="PSUM") as pr_psp:
                pw_sb = [pwpool.tile([64, C], bf16, name=f"pw_sb{hh}",
                                     tag=f"pw_sb{hh}") for hh in range(4)]
                for hh in range(4):
                    nc.sync.dma_start(out=pw_sb[hh],
                                      in_=pwT[hh * 64:(hh + 1) * 64, :])
                for mt in range(8):
                    for ch in range(NCH):
                        ps = pr_psp.tile([128, CW], f32, name="pr_ps",
                                         tag="pr_ps")
                        for hh in range(4):
                            nc.tensor.matmul(
                                ps,
                                lhsT=pw_sb[hh][:, mt * 128:(mt + 1) * 128],
                                rhs=av_sb[hh][:, ch * CW:(ch + 1) * CW],
                                start=(hh == 0), stop=(hh == 3))
                        stage = prstage.tile([128, CW], f32, name="pr_stage",
                                             tag="pr_stage")
                        nc.vector.tensor_copy(out=stage, in_=ps)
                        nc.sync.dma_start(
                            out=outT[mt * 128:(mt + 1) * 128,
                                     ch * CW:(ch + 1) * CW],
                            in_=stage)

    nc.compile()
    return nc


def _get_program(T):
    if T not in _PROG_CACHE:
        _PROG_CACHE[T] = _build_program(T)
    return _PROG_CACHE[T]


def _prep_inputs(x, attn_w, attn_b, proj_w):
    """Host-side sharding/layout prep. Returns per-core in_maps."""
    B, T, C_ = x.shape
    bf = ml_dtypes.bfloat16
    scale = 1.0 / np.sqrt(D)
    xTs = [np.ascontiguousarray(x[b].T.astype(bf)) for b in range(B)]
    in_maps = []
    for c in range(N_CORES):
        b, g = divmod(c, 4)
        r0 = 256 * g
        wq = attn_w[r0:r0 + 256] * scale
        wk = attn_w[C_ + r0:C_ + r0 + 256]
        wv = attn_w[2 * C_ + r0:2 * C_ + r0 + 256]
        wgT = np.ascontiguousarray(
            np.concatenate([wq, wk, wv], axis=0).T.astype(bf))
        bgv = np.concatenate([attn_b[r0:r0 + 256] * scale,
                              attn_b[C_ + r0:C_ + r0 + 256],
                              attn_b[2 * C_ + r0:2 * C_ + r0 + 256]])
        pwTg = np.ascontiguousarray(proj_w[:, r0:r0 + 256].T.astype(bf))
        in_maps.append({
            "xT": xTs[b],
            "wT": wgT,
            "bg": bgv.astype(np.float32),
            "pwT": pwTg,
        })
    return in_maps


def _gather(results, proj_b, B, T):
    out = np.empty((B, T, C), dtype=np.float32)
    for b in range(B):
        acc = results[4 * b]["outT"].astype(np.float32).copy()
        for g in range(1, 4):
            acc += results[4 * b + g]["outT"]
        out[b] = acc.T + proj_b[None, :]
    return out


def kernel(x, attn_w, attn_b, proj_w, proj_b, _trace=False):
    from concourse.bass_utils import run_bass_kernel_spmd
    x = np.asarray(x, dtype=np.float32)
    attn_w = np.asarray(attn_w, dtype=np.float32)
    attn_b = np.asarray(attn_b, dtype=np.float32)
    proj_w = np.asarray(proj_w, dtype=np.float32)
    proj_b = np.asarray(proj_b, dtype=np.float32)

    B, T, _ = x.shape
    nc = _get_program(T)
    in_maps = _prep_inputs(x, attn_w, attn_b, proj_w)
    res = run_bass_kernel_spmd(nc, in_maps, core_ids=list(range(N_CORES)),
                               trace=_trace)
    out = _gather(res.results, proj_b, B, T)
    if _trace:
        return out, res
    return out


# revision 14
# speedup vs baseline: 1.5164x; 1.1892x over previous
"""Causal multi-head attention (B=2, T=2048, C=1024, H=16, D=64) on 8 Trainium2
NeuronCores.

Sharding: core c = 4*b + g handles batch b (2-way data parallel) and head
group g (4-way tensor parallel over the 16 heads, 4 heads per core).  Each
core computes its heads' QKV projection, causal attention, and a partial
output projection (over its 256 feature columns); the host sums the 4
partials per batch (the "all-reduce") and adds the projection bias.

Device layout is fully transposed ("feature-major"): activations live as
[feature, token] so every matmul contraction runs over the SBUF partition
axis.  Scores are computed as S^T[s, t] = k^T.T @ q^T per head (two heads
packed into the 128 PE rows), exponentiated unnormalized (scores are tiny:
|s| < ~6, so no max-subtraction is needed), masked causally with
affine_select, then AV^T accumulates over key tiles in PSUM with a ones
column appended to V so the softmax denominator falls out of the same
matmul.  Normalization (reciprocal + PE-row-broadcast + multiply) trails
off the critical path.  Matmul operands are bf16 (cast on host / at PSUM
eviction); accumulation stays fp32 in PSUM.
"""

import numpy as np
import ml_dtypes

NUM_HEADS = 16
C = 1024
D = 64
N_CORES = 8
CW = 512   # t-chunk width (one fp32 PSUM bank)
ST = 128   # s-tile height (one partition block)

_PROG_CACHE = {}


def _build_program(T):
    import concourse.bacc as bacc
    import concourse.mybir as mybir
    import concourse.tile as tile
    from concourse.masks import make_identity

    dt = mybir.dt
    f32 = dt.float32
    f32r = dt.float32r
    bf16 = dt.bfloat16
    AF = mybir.ActivationFunctionType
    ALU = mybir.AluOpType

    NCH = T // CW   # number of t-chunks
    NT = T // ST    # number of s-tiles
    KX = C // 128   # contraction tiles for the QKV projection
    JW = 3 * 2 * 128  # per-k-tile width of the fused qkv weight slab (768)

    nc = bacc.Bacc("TRN2", target_bir_lowering=False, debug=False,
                   num_devices=N_CORES)

    xT = nc.dram_tensor("xT", [C, T], bf16, kind="ExternalInput").ap()
    wT = nc.dram_tensor("wT", [C, JW], bf16, kind="ExternalInput").ap()
    bg = nc.dram_tensor("bg", [JW], f32, kind="ExternalInput").ap()
    pwT = nc.dram_tensor("pwT", [2 * 128, C], bf16, kind="ExternalInput").ap()
    outT = nc.dram_tensor("outT", [C, T], f32, kind="ExternalOutput").ap()

    with tile.TileContext(nc) as tc:
        with (
            tc.tile_pool(name="const", bufs=1) as const,
            tc.tile_pool(name="acts", bufs=1) as acts,
            tc.tile_pool(name="ptiles", bufs=4) as ptiles,
        ):
            # ---- qkv weights first: the first matmul waits on block 0 ----
            w_sb = [const.tile([128, JW], bf16, name=f"w_sb{kc}",
                               tag=f"w_sb{kc}") for kc in range(KX)]
            for kc in range(KX):
                nc.sync.dma_start(out=w_sb[kc],
                                  in_=wT[kc * 128:(kc + 1) * 128, :])
            b_sb = const.tile([128, 6], f32, name="b_sb")
            nc.sync.dma_start(out=b_sb, in_=bg.rearrange("(m p) -> p m", p=128))
            pw_sb = [const.tile([128, C], bf16, name=f"pw_sb{kt}",
                                tag=f"pw_sb{kt}") for kt in range(2)]
            for kt in range(2):
                nc.sync.dma_start(out=pw_sb[kt],
                                  in_=pwT[kt * 128:(kt + 1) * 128, :])
            ones_f32 = const.tile([128, 64], f32, name="ones_f32")
            nc.vector.memset(ones_f32, 1.0)
            ones_bf = const.tile([128, 64], bf16, name="ones_bf")
            nc.vector.tensor_copy(out=ones_bf, in_=ones_f32)

            # persistent activations: q/k per head-pair, v (natural), av^T
            q_sb = [acts.tile([128, T], bf16, name=f"q_sb{p}", tag=f"q_sb{p}")
                    for p in range(2)]
            k_sb = [acts.tile([128, T], bf16, name=f"k_sb{p}", tag=f"k_sb{p}")
                    for p in range(2)]
            v_sb = [acts.tile([128, T], bf16,
                              name=f"v_sb{p}", tag=f"v_sb{p}")
                    for p in range(2)]
            av_sb = [acts.tile([128, T], bf16, name=f"av_sb{p}",
                               tag=f"av_sb{p}") for p in range(2)]

            # ---- phase 1: fused QKV projection (outputs transposed) ----
            with tc.tile_pool(name="xslab", bufs=16) as xslab, \
                 tc.tile_pool(name="vtsb", bufs=1) as vtsb, \
                 tc.tile_pool(name="qkv_ps", bufs=2, space="PSUM") as qkv_psp:
                vT_sb = [vtsb.tile([128, T], bf16, name=f"vT_sb{p}",
                                   tag=f"vT_sb{p}") for p in range(2)]
                qkv_dst = q_sb + k_sb + vT_sb  # m-tiles: q0 q1 k0 k1 v0 v1
                for ch in range(NCH):
                    slabs = []
                    for kc in range(KX):
                        sl = xslab.tile([128, CW], bf16, name=f"slab{kc}",
                                        tag="slab")
                        nc.sync.dma_start(
                            out=sl, in_=xT[kc * 128:(kc + 1) * 128,
                                           ch * CW:(ch + 1) * CW])
                        slabs.append(sl)
                    for mt in range(6):
                        ps = qkv_psp.tile([128, CW], f32, name="qkv_ps",
                                          tag="qkv_ps")
                        for kc in range(KX):
                            nc.tensor.matmul(
                                ps,
                                lhsT=w_sb[kc][:, mt * 128:(mt + 1) * 128],
                                rhs=slabs[kc],
                                start=(kc == 0), stop=(kc == KX - 1))
                        nc.vector.tensor_scalar_add(
                            out=qkv_dst[mt][:, ch * CW:(ch + 1) * CW],
                            in0=ps, scalar1=b_sb[:, mt:mt + 1])

                # ---- phase 1b: transpose v^T -> v via DMA xbar ----
                for p in range(2):
                    for i in range(NT):
                        nc.sync.dma_start(
                            out=v_sb[p][:, i * 128:(i + 1) * 128],
                            in_=vT_sb[p][:, i * 128:(i + 1) * 128],
                            transpose=True)

            # ---- phase 2: causal attention per head pair ----
            with tc.tile_pool(name="small", bufs=2) as small, \
                 tc.tile_pool(name="sc_ps", bufs=2, space="PSUM") as sc_psp, \
                 tc.tile_pool(name="av_ps", bufs=2, space="PSUM") as av_psp, \
                 tc.tile_pool(name="dn_ps", bufs=2, space="PSUM") as dn_psp:
                for p in range(2):
                    for ch in range(NCH):
                        n_i = 4 * ch + 4  # s-tiles 0 .. 4*ch+3
                        av_ps = av_psp.tile([128, CW], f32, name="av_ps",
                                            tag="av")
                        dn_ps = dn_psp.tile([128, CW], f32, name="dn_ps",
                                            tag="dn")
                        for i in range(n_i):
                            m = i - 4 * ch
                            col0 = 128 * m if m > 0 else 0
                            sc_ps = sc_psp.tile([128, 2 * CW], f32,
                                                name="sc_ps", tag="sc_ps")
                            for h2 in range(2):
                                nc.tensor.matmul(
                                    sc_ps[:, h2 * CW + col0:(h2 + 1) * CW],
                                    lhsT=k_sb[p][h2 * 64:(h2 + 1) * 64,
                                                 i * 128:(i + 1) * 128],
                                    rhs=q_sb[p][h2 * 64:(h2 + 1) * 64,
                                                ch * CW + col0:
                                                (ch + 1) * CW],
                                    start=True, stop=True)
                            p_sb = ptiles.tile([128, 2 * CW], bf16,
                                               name="p_sb", tag="p_sb")
                            sc3 = sc_ps.rearrange("q (h w) -> q h w", h=2)
                            p3 = p_sb.rearrange("q (h w) -> q h w", h=2)
                            nc.scalar.activation(
                                out=p3[:, :, col0:CW],
                                in_=sc3[:, :, col0:CW], func=AF.Exp)
                            if m >= 0:
                                blk = p3[:, :, col0:col0 + 128]
                                nc.gpsimd.affine_select(
                                    out=blk, in_=blk,
                                    pattern=[[0, 2], [1, 128]],
                                    channel_multiplier=-1, base=0,
                                    compare_op=ALU.is_ge, fill=0.0)
                            last = (i == n_i - 1)
                            for h2 in range(2):
                                nc.tensor.matmul(
                                    av_ps[h2 * 64:(h2 + 1) * 64, col0:CW],
                                    lhsT=v_sb[p][:, i * 128 + h2 * 64:
                                                 i * 128 + (h2 + 1) * 64],
                                    rhs=p_sb[:, h2 * CW + col0:
                                             (h2 + 1) * CW],
                                    start=(i == 0), stop=last,
                                    skip_group_check=True)
                            for h2 in range(2):
                                nc.tensor.matmul(
                                    dn_ps[h2 * 64:(h2 + 1) * 64, col0:CW],
                                    lhsT=ones_bf,
                                    rhs=p_sb[:, h2 * CW + col0:
                                             (h2 + 1) * CW],
                                    start=(i == 0), stop=last,
                                    skip_group_check=True)
                        # epilogue: evict av to SBUF fast (frees PSUM);
                        # one reciprocal over the row-broadcast denominators
                        # and the normalize multiply trail off the critical
                        # path.
                        av_un = small.tile([128, CW], f32, name="av_un",
                                           tag="av_un", bufs=2)
                        nc.vector.tensor_copy(out=av_un, in_=av_ps)
                        rc = small.tile([128, CW], f32, name="rc",
                                        tag="rc", bufs=2)
                        with nc.allow_low_precision(
                                reason="softmax denominator reciprocal"):
                            nc.vector.reciprocal(out=rc, in_=dn_ps)
                        nc.vector.tensor_mul(
                            av_sb[p][:, ch * CW:(ch + 1) * CW],
                            av_un, rc)

            # ---- phase 3: partial output projection ----
            with tc.tile_pool(name="prstage", bufs=4) as prstage, \
                 tc.tile_pool(name="pr_ps", bufs=4, space="PSUM") as pr_psp:
                for mt in range(8):
                    for ch in range(NCH):
                        ps = pr_psp.tile([128, CW], f32, name="pr_ps",
                                         tag="pr_ps")
                        for kt in range(2):
                            nc.tensor.matmul(
                                ps,
                                lhsT=pw_sb[kt][:, mt * 128:(mt + 1) * 128],
                                rhs=av_sb[kt][:, ch * CW:(ch + 1) * CW],
                                start=(kt == 0), stop=(kt == 1))
                        stage = prstage.tile([128, CW], f32, name="pr_stage",
                                             tag="pr_stage")
                        nc.vector.tensor_copy(out=stage, in_=ps)
                        nc.sync.dma_start(
                            out=outT[mt * 128:(mt + 1) * 128,
                                     ch * CW:(ch + 1) * CW],
                            in_=stage)

    nc.compile()
    return nc


def _get_program(T):
    if T not in _PROG_CACHE:
        _PROG_CACHE[T] = _build_program(T)
    return _PROG_CACHE[T]


def _prep_inputs(x, attn_w, attn_b, proj_w):
    """Host-side sharding/layout prep. Returns per-core in_maps."""
    B, T, C_ = x.shape
    bf = ml_dtypes.bfloat16
    scale = 1.0 / np.sqrt(D)
    xTs = [np.ascontiguousarray(x[b].T.astype(bf)) for b in range(B)]
    in_maps = []
    for c in range(N_CORES):
        b, g = divmod(c, 4)
        r0 = 256 * g
        wq = attn_w[r0:r0 + 256] * scale
        wk = attn_w[C_ + r0:C_ + r0 + 256]
        wv = attn_w[2 * C_ + r0:2 * C_ + r0 + 256]
        wgT = np.ascontiguousarray(
            np.concatenate([wq, wk, wv], axis=0).T.astype(bf))
        bgv = np.concatenate([attn_b[r0:r0 + 256] * scale,
                              attn_b[C_ + r0:C_ + r0 + 256],
                              attn_b[2 * C_ + r0:2 * C_ + r0 + 256]])
        pwTg = np.ascontiguousarray(proj_w[:, r0:r0 + 256].T.astype(bf))
        in_maps.append({
            "xT": xTs[b],
            "wT": wgT,
            "bg": bgv.astype(np.float32),
            "pwT": pwTg,
        })
    return in_maps


def _gather(results, proj_b, B, T):
    out = np.empty((B, T, C), dtype=np.float32)
    for b in range(B):
        acc = results[4 * b]["outT"].astype(np.float32).copy()
        for g in range(1, 4):
            acc += results[4 * b + g]["outT"]
        out[b] = acc.T + proj_b[None, :]
    return out


def kernel(x, attn_w, attn_b, proj_w, proj_b, _trace=False):
    from concourse.bass_utils import run_bass_kernel_spmd
    x = np.asarray(x, dtype=np.float32)
    attn_w = np.asarray(attn_w, dtype=np.float32)
    attn_b = np.asarray(attn_b, dtype=np.float32)
    proj_w = np.asarray(proj_w, dtype=np.float32)
    proj_b = np.asarray(proj_b, dtype=np.float32)

    B, T, _ = x.shape
    nc = _get_program(T)
    in_maps = _prep_inputs(x, attn_w, attn_b, proj_w)
    res = run_bass_kernel_spmd(nc, in_maps, core_ids=list(range(N_CORES)),
                               trace=_trace)
    out = _gather(res.results, proj_b, B, T)
    if _trace:
        return out, res
    return out
